# revision 7
# baseline (speedup 1.0000x reference)
"""Guided filter (r=40, eps=1e-3) on 8 Trainium2 NeuronCores.

Sharding: pure data-parallel over the batch dim (8 batches -> 8 cores).
Each core processes 3 channel-images of 512x512.

Hybrid box2d per tensor (1 matmul pass + 1 DVE scan instead of 2 matmul
passes):
  - W-direction box: one tensor_tensor_scan along the free dim computes
    the 81-wide running window sum exactly in fp32
    (state = state + x[t] - x[t-81]), over a zero-padded layout that
    self-drains at chunk boundaries. Output downcast to bf16.
  - H-direction box: banded matmul on TensorE (data stationary, band
    moving) which also transposes the layout, with bf16(1/n) in the band
    columns so the output is box2d(X)/n_h.
  Elementwise stage in [w-part, h-free] layout; per-partition 1/n_w
  scalars fold the w-normalization; eps folds in via a rank-1 matmul.
  Stage3 (a, b) runs the same hybrid in the transposed layout, landing
  back in [h, w] for the final combine.
"""

import os
import sys
import numpy as np
import ml_dtypes
from contextlib import ExitStack

sys.path.insert(0, "/opt/trn_rl_repo")

import concourse.bass as bass
import concourse.tile as tile
from concourse import bacc, mybir
from concourse.bass_utils import run_bass_kernel_spmd

F32 = mybir.dt.float32
BF16 = mybir.dt.bfloat16
ALU = mybir.AluOpType

R = 40
EPS = 1e-3
HW_ = 512
NB = 4          # 128-row blocks per axis
CH = 3
P = 128
NCORES = 8

K = 2 * R + 1   # 81
LEAD = K        # leading zero pad per segment
SEG = LEAD + HW_              # 593
TOTW = NB * SEG + R           # 2412 padded tile width
SCANL = TOTW - LEAD           # 2331 scan length
OFF = R                       # box out offset within scan output


def _band_range(c):
    n0 = max(0, P * c - R)
    n1 = min(HW_, P * c + P + R)
    return n0, n1


def make_consts():
    idx = np.arange(HW_)
    n1d = (np.minimum(idx + R, HW_ - 1) - np.maximum(idx - R, 0) + 1).astype(np.float64)
    inv_n = 1.0 / n1d

    mask = (np.abs(idx[:, None] - idx[None, :]) <= R)
    bandH = (mask * inv_n[None, :]).astype(ml_dtypes.bfloat16)
    # rows scaled by inw(w_in) for the b-tensor stage3 pass
    bandWb = (mask * (inv_n[:, None] * inv_n[None, :])).astype(ml_dtypes.bfloat16)
    # [512k, 512n] -> [128 kp, 4*512 (kb, n)]
    def blk(b):
        return np.ascontiguousarray(
            b.reshape(NB, P, HW_).transpose(1, 0, 2).reshape(P, NB * HW_))
    bandH = blk(bandH)
    bandWb = blk(bandWb)

    invn = np.ascontiguousarray(inv_n.reshape(NB, P).T).astype(np.float32)  # [128,4]
    epsnw = (EPS * n1d).astype(ml_dtypes.bfloat16).reshape(1, HW_)
    ones = np.ones((1, HW_), dtype=ml_dtypes.bfloat16)
    return {"bandH": bandH, "bandWb": bandWb, "invn": invn,
            "invnn": -invn, "epsnw": epsnw, "ones": ones}


def _img_view(dram_ap, c):
    # [3, 512, 512] DRAM tensor -> channel c as [128 hp, 4 hb, 512 w]
    return dram_ap[c].rearrange("(hb hp) w -> hp hb w", hp=P)


def _data_view(t):
    # padded tile [128, 2412] -> [128, 4, 512] data regions
    return t[:, 0:NB * SEG].rearrange("p (b x) -> p b x", x=SEG)[:, :, LEAD:SEG]


def build_model():
    nc = bacc.Bacc("TRN2", target_bir_lowering=False, debug=False,
                   num_devices=NCORES)
    I_d = nc.dram_tensor("I", [CH, HW_, HW_], F32, kind="ExternalInput").ap()
    p_d = nc.dram_tensor("p", [CH, HW_, HW_], F32, kind="ExternalInput").ap()
    bandH_d = nc.dram_tensor("bandH", [P, NB * HW_], BF16, kind="ExternalInput").ap()
    bandWb_d = nc.dram_tensor("bandWb", [P, NB * HW_], BF16, kind="ExternalInput").ap()
    invn_d = nc.dram_tensor("invn", [P, NB], F32, kind="ExternalInput").ap()
    invnn_d = nc.dram_tensor("invnn", [P, NB], F32, kind="ExternalInput").ap()
    epsnw_d = nc.dram_tensor("epsnw", [1, HW_], BF16, kind="ExternalInput").ap()
    ones_d = nc.dram_tensor("ones", [1, HW_], BF16, kind="ExternalInput").ap()
    out_d = nc.dram_tensor("out", [CH, HW_, HW_], F32, kind="ExternalOutput").ap()

    with tile.TileContext(nc) as tc:
        with ExitStack() as ctx:
            build_kernel(ctx, tc, I_d, p_d, out_d,
                         bandH_d, bandWb_d, invn_d, invnn_d, epsnw_d, ones_d)
    nc.compile()
    return nc


def build_kernel(ctx, tc, I_d, p_d, out_d, bandH_d, bandWb_d, invn_d,
                 invnn_d, epsnw_d, ones_d):
    nc = tc.nc

    consts = ctx.enter_context(tc.tile_pool(name="consts", bufs=1))
    bandH = consts.tile_from(bandH_d)
    bandWb = consts.tile_from(bandWb_d)
    invn = consts.tile_from(invn_d)
    invnn = consts.tile_from(invnn_d)
    epsnw = consts.tile_from(epsnw_d)
    ones = consts.tile_from(ones_d)
    bH = bandH[:].rearrange("p (b n) -> p b n", n=HW_)
    bWb = bandWb[:].rearrange("p (b n) -> p b n", n=HW_)

    # padded source tiles (pads zeroed once per buffer in the prologue)
    pIn = ctx.enter_context(tc.tile_pool(name="inp", bufs=2))
    pDer = ctx.enter_context(tc.tile_pool(name="der", bufs=2))
    pScan = ctx.enter_context(tc.tile_pool(name="scn", bufs=2))
    pAB = ctx.enter_context(tc.tile_pool(name="ab", bufs=2))
    pSab = ctx.enter_context(tc.tile_pool(name="sab", bufs=2))
    pS2 = ctx.enter_context(tc.tile_pool(name="st2", bufs=2))
    pOut = ctx.enter_context(tc.tile_pool(name="outp", bufs=2))
    pPs = ctx.enter_context(tc.tile_pool(name="ps1", bufs=1, space="PSUM"))
    pPs3 = ctx.enter_context(tc.tile_pool(name="ps3", bufs=2, space="PSUM"))

    def padded(pool, tag, dtype):
        return pool.tile([P, TOTW], dtype, tag=tag, name=tag)

    PAD_TAGS = [(pIn, "If", F32), (pIn, "pf", F32),
                (pDer, "Ip", BF16), (pDer, "II", BF16),
                (pAB, "a", BF16), (pAB, "b", BF16)]

    # prologue: zero the pads of every rotating buffer
    for _ in range(2):
        for pool, tag, dt in PAD_TAGS:
            t = padded(pool, tag, dt)
            pads = t[:, 0:NB * SEG].rearrange("p (b x) -> p b x", x=SEG)[:, :, 0:LEAD]
            nc.gpsimd.memset(pads, 0.0)
            nc.gpsimd.memset(t[:, NB * SEG:TOTW], 0.0)

    def scan_box(pool, tag, src):
        s = pool.tile([P, SCANL], BF16, tag=tag, name=tag)
        nc.vector.tensor_tensor_scan(
            s[:], src[:, LEAD:TOTW], src[:, 0:SCANL], 0.0,
            op0=ALU.add, op1=ALU.subtract)
        return s

    def boxpass_mm(S, band, q, i, extra=None):
        """Banded matmuls: box over partition axis of the (virtual) [h,w]
        image whose scan output is S; output chunk i -> psum q."""
        for j in range(NB):
            n0, n1 = _band_range(j)
            last = (j == NB - 1) and extra is None
            nc.tensor.matmul(
                q[:, n0:n1],
                lhsT=S[:, SEG * j + OFF + P * i: SEG * j + OFF + P * i + P],
                rhs=band[:, j, n0:n1],
                start=(j == 0), stop=last)
        if extra is not None:
            nc.tensor.matmul(
                q[:, :], lhsT=extra[:1, P * i:P * (i + 1)], rhs=ones[:1, :],
                start=False, stop=True)

    for c in range(CH):
        I_f = padded(pIn, "If", F32)
        p_f = padded(pIn, "pf", F32)
        nc.sync.dma_start(_data_view(I_f), _img_view(I_d, c))
        nc.sync.dma_start(_data_view(p_f), _img_view(p_d, c))

        Ip = padded(pDer, "Ip", BF16)
        II = padded(pDer, "II", BF16)
        nc.gpsimd.tensor_tensor(_data_view(Ip), _data_view(I_f),
                                _data_view(p_f), op=ALU.mult)
        nc.scalar.square(_data_view(II), _data_view(I_f))

        S_I = scan_box(pScan, "sI", I_f)
        S_p = scan_box(pScan, "sp", p_f)
        S_Ip = scan_box(pScan, "sIp", Ip)
        S_II = scan_box(pScan, "sII", II)

        a_t = padded(pAB, "a", BF16)
        b_t = padded(pAB, "b", BF16)
        for i in range(NB):
            qI = pPs.tile([P, HW_], F32, tag="qI")
            qp = pPs.tile([P, HW_], F32, tag="qp")
            qIp = pPs.tile([P, HW_], F32, tag="qIp")
            qII = pPs.tile([P, HW_], F32, tag="qII")
            boxpass_mm(S_I, bH, qI, i)
            boxpass_mm(S_p, bH, qp, i)
            boxpass_mm(S_Ip, bH, qIp, i)
            boxpass_mm(S_II, bH, qII, i, extra=epsnw)

            s = invn[:, i:i + 1]
            sn = invnn[:, i:i + 1]
            PIb = pS2.tile([P, HW_], BF16, tag="PIb")
            PPb = pS2.tile([P, HW_], BF16, tag="PPb")
            nc.scalar.copy(PIb[:], qI[:])
            nc.scalar.copy(PPb[:], qp[:])
            u = pS2.tile([P, HW_], BF16, tag="u")
            ncov = pS2.tile([P, HW_], BF16, tag="ncov")
            sq = pS2.tile([P, HW_], BF16, tag="sq")
            nden = pS2.tile([P, HW_], F32, tag="nden")
            rcp = pS2.tile([P, HW_], F32, tag="rcp")
            t1 = pS2.tile([P, HW_], BF16, tag="t1")
            nc.vector.scalar_tensor_tensor(
                u[:], PIb[:], 0.0, PPb[:], op0=ALU.bypass, op1=ALU.mult)
            nc.vector.scalar_tensor_tensor(
                ncov[:], u[:], sn, qIp[:], op0=ALU.mult, op1=ALU.add)
            nc.vector.scalar_tensor_tensor(
                sq[:], PIb[:], 0.0, PIb[:], op0=ALU.bypass, op1=ALU.mult)
            nc.vector.scalar_tensor_tensor(
                nden[:], sq[:], sn, qII[:], op0=ALU.mult, op1=ALU.add)
            nc.vector.reciprocal_approx_fast(rcp[:], nden[:])
            asl = a_t[:, SEG * i + LEAD: SEG * i + LEAD + HW_]
            bsl = b_t[:, SEG * i + LEAD: SEG * i + LEAD + HW_]
            nc.vector.scalar_tensor_tensor(
                asl, ncov[:], 0.0, rcp[:], op0=ALU.bypass, op1=ALU.mult)
            nc.vector.scalar_tensor_tensor(
                t1[:], asl, 0.0, PIb[:], op0=ALU.bypass, op1=ALU.mult)
            nc.vector.scalar_tensor_tensor(
                bsl, PPb[:], 0.0, t1[:], op0=ALU.bypass, op1=ALU.subtract)

        S_a = scan_box(pSab, "sa", a_t)
        S_b = scan_box(pSab, "sb", b_t)

        out_t = pOut.tile([P, NB * HW_], F32, tag="out")
        ov = out_t[:].rearrange("p (b x) -> p b x", x=HW_)
        for m in range(NB):
            ra = pPs3.tile([P, HW_], F32, tag="ra")
            rb = pPs3.tile([P, HW_], F32, tag="rb")
            boxpass_mm(S_a, bH, ra, m)
            boxpass_mm(S_b, bWb, rb, m)
            s = invn[:, m:m + 1]
            isl = I_f[:, SEG * m + LEAD: SEG * m + LEAD + HW_]
            tt = pS2.tile([P, HW_], F32, tag="tt")
            sm = pS2.tile([P, HW_], F32, tag="sm")
            nc.vector.scalar_tensor_tensor(
                tt[:], isl, 0.0, ra[:], op0=ALU.bypass, op1=ALU.mult)
            nc.vector.scalar_tensor_tensor(
                sm[:], tt[:], 0.0, rb[:], op0=ALU.bypass, op1=ALU.add)
            nc.scalar.mul(ov[:, m, :], sm[:], s)

        nc.sync.dma_start(_img_view(out_d, c), ov)


_NC_CACHE = None
LAST_RESULT = None


def _get_model():
    global _NC_CACHE
    if _NC_CACHE is None:
        _NC_CACHE = build_model()
    return _NC_CACHE


def kernel(I, p):
    global LAST_RESULT
    I = np.asarray(I, dtype=np.float32)
    p = np.asarray(p, dtype=np.float32)
    B = I.shape[0]
    assert I.shape == (B, CH, HW_, HW_), I.shape
    nc = _get_model()
    consts = make_consts()
    in_maps = []
    for k in range(NCORES):
        m = {"I": np.ascontiguousarray(I[k]), "p": np.ascontiguousarray(p[k])}
        m.update(consts)
        in_maps.append(m)
    kwargs = {}
    if os.environ.get("BASS_TRACE_DIR"):
        kwargs["tmpdir"] = os.environ["BASS_TRACE_DIR"]
    res = run_bass_kernel_spmd(nc, in_maps, core_ids=list(range(NCORES)), **kwargs)
    LAST_RESULT = res
    out = np.stack([res.results[k]["out"] for k in range(NCORES)], axis=0)
    return out.astype(np.float32)


if __name__ == "__main__":
    rng = np.random.default_rng(0)
    I = rng.random((8, CH, HW_, HW_), dtype=np.float32)
    p = rng.random((8, CH, HW_, HW_), dtype=np.float32)
    out = kernel(I, p)
    print("out", out.shape, out.dtype, float(out.mean()))
